# revision 1
# baseline (speedup 1.0000x reference)
"""AttentionSequencePoolingLayer (DIN-style) kernel for Trainium2, 8 cores.

Reference, per batch row b (W = [Wq; Wk], each [64, 1]):
    score_t = tanh(keys_b[t] @ Wk + (query_b @ Wq + bias))
    logits  = where(t < keys_length_b, score_t, MASK_PAD)
    out_b   = softmax(logits) @ keys_b
Masking here: e = exp((score+30)*mask - 30); masked lanes give exp(-30),
which vanishes next to real weights and reproduces the reference's
uniform-softmax behaviour when every position is masked (keys_length==0).

Sharding: pure data parallel, batch 4096 -> 8 NeuronCores x 512.

Design, driven by measured TRN2 facts (this toolchain):
  - Only the natural [b, (t c)] keys DMA reaches full HBM rate (~320 GB/s);
    transposed layouts run 2-4x slower, which rules out every TensorE
    matmul formulation (PE contracts the partition dim = batch here).
  - So both contractions run on VectorE in bf16 (tensor_tensor at 2x with
    step-1 innermost APs) with pairwise fold trees replacing tensor_reduce
    (always 1x); each tree stops at width 4 and finishes with one strided
    f32 tensor_reduce (cheaper than the last two folds plus a compact, and
    more accurate). A stride-0 operand drops TT to 1x, so e is pre-expanded
    along c on ScalarE, which also does the f32->bf16 keys convert, tanh,
    exp (with fused sum) and the 1/s scaling.
  - GpSimd runs nothing: its SBUF port is lock-shared with VectorE and its
    queue would serialize with DMAs (both re-measured as net losses).
  - keys tiles are triple-buffered: each tile's bf16 keys are read by both
    the score product (early) and the output product (late), so with only
    two buffers the loads have zero slack.

Per 128-batch tile, pipelined across tiles by the Tile framework, in
t-chunks of (64, 64, 72) so DMA/ScalarE/VectorE interleave finely. The
next tile's loads+converts are emitted ahead of the current tile's
output phase so ScalarE serves them before the expansions (the converts
gate the next tile's VectorE work):
  sync DMA f32 chunk -> ACT convert to bf16 -> DVE prod = keys*Wk(bcast)
  -> DVE c-fold tree -> ACT tanh(kdot+qdot+b) -> mask/exp/normalize
  -> ACT expand e -> DVE p2 = keys*e -> DVE t-fold tree -> join -> DMA out.
"""

import sys

sys.path.insert(0, "/opt/trn_rl_repo")

import numpy as np

import concourse.bass as bass
import concourse.tile as tile
from concourse import bacc, mybir
from concourse.bass_utils import run_bass_kernel_spmd

F32 = mybir.dt.float32
BF16 = mybir.dt.bfloat16
I32 = mybir.dt.int32

B_FULL = 4096
N_CORES = 8
B = B_FULL // N_CORES  # 512
T = 200
C = 64
P = 128
N_TILES = B // P  # 4

_NC_CACHE = {}


def build_kernel():
    nc = bacc.Bacc("TRN2", target_bir_lowering=False, debug=False)

    q_d = nc.dram_tensor("queries", [B, 1, C], F32, kind="ExternalInput").ap()
    k_d = nc.dram_tensor("keys", [B, T, C], F32, kind="ExternalInput").ap()
    kl_d = nc.dram_tensor("keys_length", [B, 1], I32, kind="ExternalInput").ap()
    w_d = nc.dram_tensor("W", [2 * C, 1], F32, kind="ExternalInput").ap()
    b_d = nc.dram_tensor("b", [1], F32, kind="ExternalInput").ap()
    out_d = nc.dram_tensor("out", [B, 1, C], F32, kind="ExternalOutput").ap()

    with tile.TileContext(nc) as tc:
        with (
            tc.tile_pool(name="const", bufs=1) as cpool,
            tc.tile_pool(name="kf32", bufs=2) as fpool,
            tc.tile_pool(name="keys", bufs=3) as kpool,
            tc.tile_pool(name="prod", bufs=1) as ppool,
            tc.tile_pool(name="p2p", bufs=1) as p2pool,
            tc.tile_pool(name="ex", bufs=1) as xpool,
            tc.tile_pool(name="small", bufs=2) as spool,
            tc.tile_pool(name="ps", bufs=1, space="PSUM") as ps,
        ):
            # ---- setup: broadcast W row + bias to all partitions ----
            wrow = cpool.tile([1, 2 * C + 1], F32)
            nc.sync.dma_start(wrow[:, 0 : 2 * C], w_d.rearrange("c o -> o c"))
            nc.sync.dma_start(wrow[:, 2 * C : 2 * C + 1], b_d.unsqueeze(0))
            ones_col = cpool.tile([1, P], F32)
            nc.vector.memset(ones_col[:], 1.0)
            wbc_ps = ps.tile([P, 2 * C + 1], F32)
            nc.tensor.matmul(wbc_ps[:], ones_col[:], wrow[:], start=True, stop=True)
            wbc = cpool.tile([P, 2 * C + 1], F32)
            nc.vector.tensor_copy(wbc[:], wbc_ps[:])
            wq_bc = wbc[:, 0:C]
            bias_bc = wbc[:, 2 * C : 2 * C + 1]
            wk_bf = cpool.tile([P, C], BF16)
            nc.vector.tensor_copy(wk_bf[:], wbc[:, C : 2 * C])

            iota_i = cpool.tile([P, T], I32)
            nc.gpsimd.iota(iota_i[:], pattern=[[1, T]], base=0, channel_multiplier=0)
            iota_f = cpool.tile([P, T], F32)
            nc.vector.tensor_copy(iota_f[:], iota_i[:])
            neg30 = cpool.tile([P, 1], F32)
            nc.vector.memset(neg30[:], -30.0)

            CH = ((0, 64), (64, 128), (128, T))

            def emit_load(i):
                sl = slice(i * P, (i + 1) * P)
                kbf = kpool.tile([P, T * C], BF16, tag="kbf")
                for t0, t1 in CH:
                    kfh = fpool.tile([P, 72 * C], F32, tag="kfh")
                    nc.sync.dma_start(
                        kfh[:, 0 : (t1 - t0) * C],
                        k_d[sl, t0:t1, :].rearrange("b t c -> b (t c)"),
                    )
                    nc.scalar.copy(
                        kbf[:, t0 * C : t1 * C], kfh[:, 0 : (t1 - t0) * C]
                    )
                q_t = spool.tile([P, C], F32, tag="q_t")
                nc.sync.dma_start(q_t[:], q_d[sl, 0, :])
                kl_t = spool.tile([P, 1], I32, tag="kl_t")
                nc.sync.dma_start(kl_t[:], kl_d[sl])
                kl_f = spool.tile([P, 1], F32, tag="kl_f")
                nc.vector.tensor_copy(kl_f[:], kl_t[:])
                mask = spool.tile([P, T], F32, tag="mask")
                nc.vector.tensor_scalar(
                    mask[:], iota_f[:], kl_f[:], None, op0=mybir.AluOpType.is_lt
                )
                qprod = spool.tile([P, C], F32, tag="qprod")
                nc.vector.tensor_tensor(
                    qprod[:], q_t[:], wq_bc, op=mybir.AluOpType.mult
                )
                qdot = spool.tile([P, 1], F32, tag="qdot")
                nc.vector.reduce_sum(qdot[:], qprod[:], axis=mybir.AxisListType.X)
                qdotb = spool.tile([P, 1], F32, tag="qdotb")
                nc.vector.tensor_tensor(
                    qdotb[:], qdot[:], bias_bc, op=mybir.AluOpType.add
                )
                return kbf, mask, qdotb

            loads = {0: emit_load(0)}
            for i in range(N_TILES):
                sl = slice(i * P, (i + 1) * P)
                kbf, mask, qdotb = loads.pop(i)
                k3 = kbf[:].rearrange("p (t c) -> p t c", t=T, c=C)
                prod = ppool.tile([P, T * C], BF16, tag="prod")
                p3 = prod[:].rearrange("p (t c) -> p t c", t=T, c=C)
                kdot = spool.tile([P, T], F32)
                # scores: product + c-folds to width 4, then a strided
                # reduce straight into f32 kdot
                for t0, t1 in CH:
                    nc.vector.tensor_tensor(
                        p3[:, t0:t1, :],
                        k3[:, t0:t1, :],
                        wk_bf[:].unsqueeze(1).to_broadcast((P, t1 - t0, C)),
                        op=mybir.AluOpType.mult,
                    )
                    w_ = C // 2
                    while w_ >= 4:
                        nc.vector.tensor_tensor(
                            p3[:, t0:t1, 0:w_],
                            p3[:, t0:t1, 0:w_],
                            p3[:, t0:t1, w_ : 2 * w_],
                            op=mybir.AluOpType.add,
                        )
                        w_ //= 2
                    nc.vector.reduce_sum(
                        kdot[:, t0:t1],
                        p3[:, t0:t1, 0:4],
                        axis=mybir.AxisListType.X,
                    )

                score = spool.tile([P, T], F32)
                nc.scalar.activation(
                    score[:],
                    kdot[:],
                    mybir.ActivationFunctionType.Tanh,
                    bias=qdotb[:],
                    scale=1.0,
                )
                sm = spool.tile([P, T], F32)
                nc.vector.scalar_tensor_tensor(
                    sm[:],
                    score[:],
                    30.0,
                    mask[:],
                    op0=mybir.AluOpType.add,
                    op1=mybir.AluOpType.mult,
                )
                e = spool.tile([P, T], F32)
                ssum = spool.tile([P, 1], F32)
                nc.scalar.activation(
                    e[:],
                    sm[:],
                    mybir.ActivationFunctionType.Exp,
                    bias=neg30[:],
                    scale=1.0,
                    accum_out=ssum[:],
                )
                rs = spool.tile([P, 1], F32)
                nc.vector.reciprocal(rs[:], ssum[:])
                # en = e / s (bf16), via ACT with per-partition scale
                en = spool.tile([P, T], BF16)
                nc.scalar.activation(
                    en[:],
                    e[:],
                    mybir.ActivationFunctionType.Copy,
                    bias=0.0,
                    scale=rs[:],
                )

                # hoist next tile's loads+converts ahead of the output phase
                # so ScalarE serves them before this tile's expansions
                if i + 1 < N_TILES:
                    loads[i + 1] = emit_load(i + 1)

                # ---- output: per chunk expand -> multiply; then t-folds ----
                enx = xpool.tile([P, T * C], BF16, tag="enx")
                enx3 = enx[:].rearrange("p (t c) -> p t c", t=T, c=C)
                en3 = en[:].unsqueeze(2).to_broadcast((P, T, C))
                p2 = p2pool.tile([P, T * C], BF16, tag="p2")
                p23 = p2[:].rearrange("p (t c) -> p t c", t=T, c=C)
                for t0, t1 in CH:
                    nc.scalar.copy(enx3[:, t0:t1, :], en3[:, t0:t1, :])
                    nc.vector.tensor_tensor(
                        p23[:, t0:t1, :],
                        k3[:, t0:t1, :],
                        enx3[:, t0:t1, :],
                        op=mybir.AluOpType.mult,
                    )
                # fold 200 -> 8 rows, then one strided (c, t) reduce
                nc.vector.tensor_tensor(
                    p23[:, 0:72, :], p23[:, 0:72, :], p23[:, 128:T, :],
                    op=mybir.AluOpType.add,
                )
                w_ = 64
                while w_ >= 4:
                    nc.vector.tensor_tensor(
                        p23[:, 0:w_, :],
                        p23[:, 0:w_, :],
                        p23[:, w_ : 2 * w_, :],
                        op=mybir.AluOpType.add,
                    )
                    w_ //= 2
                out_t = spool.tile([P, C], F32)
                nc.vector.reduce_sum(
                    out_t[:],
                    p2[:].rearrange("p (t c) -> p c t", t=T, c=C)[:, :, 0:4],
                    axis=mybir.AxisListType.X,
                )
                nc.sync.dma_start(out_d[sl, 0, :], out_t[:])

    nc.compile()
    return nc


def get_kernel():
    if "nc" not in _NC_CACHE:
        _NC_CACHE["nc"] = build_kernel()
    return _NC_CACHE["nc"]


def kernel(queries, keys, keys_length, W, b, **run_kwargs):
    nc = get_kernel()
    in_maps = []
    for c in range(N_CORES):
        sl = slice(c * B, (c + 1) * B)
        in_maps.append(
            {
                "queries": np.ascontiguousarray(queries[sl], dtype=np.float32),
                "keys": np.ascontiguousarray(keys[sl], dtype=np.float32),
                "keys_length": np.ascontiguousarray(keys_length[sl], dtype=np.int32),
                "W": np.ascontiguousarray(W, dtype=np.float32),
                "b": np.ascontiguousarray(b, dtype=np.float32),
            }
        )
    res = run_bass_kernel_spmd(nc, in_maps, core_ids=list(range(N_CORES)), **run_kwargs)
    out = np.concatenate([res.results[c]["out"] for c in range(N_CORES)], axis=0)
    if run_kwargs:
        kernel.last_result = res
    return out



# revision 2
# speedup vs baseline: 1.0078x; 1.0078x over previous
"""AttentionSequencePoolingLayer (DIN-style) kernel for Trainium2, 8 cores. v4.

Reference, per batch row b (W = [Wq; Wk], each [64, 1]):
    score_t = tanh(keys_b[t] @ Wk + (query_b @ Wq + bias))
    logits  = where(t < keys_length_b, score_t, MASK_PAD)
    out_b   = softmax(logits) @ keys_b

v4 = v3 (pair-interleaved keys; both DVE products at 2x; no ACT expand)
with a software-pipelined DVE emission order. v3 measured DVE busy 125.5us
against a 160us wall: the tanh->stt->exp->recip->en ping-pong with ACT left
~2-3us/tile of DVE idle, and head/tail overlap was poor. v4's steady-state
DVE stream per tile i:
    stt_i | smalls_{i+1} | recip_i | score_{i+1} | prod2_i | tfold_i
so the ACT softmax hops overlap next-tile DVE work, and ACT runs
    tanh_i, exp_i, en_i, conv_{i+1} x3, tanh_{i+1} ...
with converts filling ACT's wait on the next kdot. Keys chunk 2 loads on the
scalar HWDGE ring to parallelize DMA dispatch.

Engines measured (v3): DVE 125us busy = sole bottleneck; ACT 67us; DMA 89us;
gpsimd offload of folds re-measured as a big net loss (gpsimd TT ~5x slower
than DVE and port contention inflates DVE TTs 1.5x) - do not use gpsimd.
tensor_tensor_reduce crashes HW - do not use.

Sharding: pure data parallel, batch 4096 -> 8 NeuronCores x 512.
"""

import sys

sys.path.insert(0, "/opt/trn_rl_repo")

import numpy as np

import concourse.bass as bass
import concourse.tile as tile
from concourse import bacc, mybir
from concourse.bass_utils import run_bass_kernel_spmd

F32 = mybir.dt.float32
BF16 = mybir.dt.bfloat16
I32 = mybir.dt.int32

B_FULL = 4096
N_CORES = 8
B = B_FULL // N_CORES  # 512
T = 200
T2 = T // 2
C = 64
P = 128
N_TILES = B // P  # 4

_NC_CACHE = {}


def build_kernel():
    nc = bacc.Bacc("TRN2", target_bir_lowering=False, debug=False)

    q_d = nc.dram_tensor("queries", [B, 1, C], F32, kind="ExternalInput").ap()
    k_d = nc.dram_tensor("keys", [B, T, C], F32, kind="ExternalInput").ap()
    kl_d = nc.dram_tensor("keys_length", [B, 1], I32, kind="ExternalInput").ap()
    w_d = nc.dram_tensor("W", [2 * C, 1], F32, kind="ExternalInput").ap()
    b_d = nc.dram_tensor("b", [1], F32, kind="ExternalInput").ap()
    out_d = nc.dram_tensor("out", [B, 1, C], F32, kind="ExternalOutput").ap()

    with tile.TileContext(nc) as tc:
        with (
            tc.tile_pool(name="const", bufs=1) as cpool,
            tc.tile_pool(name="kf32", bufs=2) as fpool,
            tc.tile_pool(name="keys", bufs=3) as kpool,
            tc.tile_pool(name="prod", bufs=1) as ppool,
            tc.tile_pool(name="p2p", bufs=1) as p2pool,
            tc.tile_pool(name="small", bufs=2) as spool,
            tc.tile_pool(name="ps", bufs=1, space="PSUM") as ps,
        ):
            # ---- setup: broadcast W row + bias to all partitions ----
            wrow = cpool.tile([1, 2 * C + 1], F32)
            nc.sync.dma_start(wrow[:, 0 : 2 * C], w_d.rearrange("c o -> o c"))
            nc.sync.dma_start(wrow[:, 2 * C : 2 * C + 1], b_d.unsqueeze(0))
            ones_col = cpool.tile([1, P], F32)
            nc.vector.memset(ones_col[:], 1.0)
            wbc_ps = ps.tile([P, 2 * C + 1], F32)
            nc.tensor.matmul(wbc_ps[:], ones_col[:], wrow[:], start=True, stop=True)
            wbc = cpool.tile([P, 2 * C + 1], F32)
            nc.vector.tensor_copy(wbc[:], wbc_ps[:])
            wq_bc = wbc[:, 0:C]
            bias_bc = wbc[:, 2 * C : 2 * C + 1]
            # wkp[p, (c two)] = wk[c] as bf16 pairs for prod1
            wkp = cpool.tile([P, 2 * C], BF16)
            nc.scalar.copy(
                wkp[:].rearrange("p (c two) -> p c two", c=C, two=2),
                wbc[:, C : 2 * C].unsqueeze(2).to_broadcast((P, C, 2)),
            )

            iota_i = cpool.tile([P, T], I32)
            nc.gpsimd.iota(iota_i[:], pattern=[[1, T]], base=0, channel_multiplier=0)
            iota_f = cpool.tile([P, T], F32)
            nc.vector.tensor_copy(iota_f[:], iota_i[:])
            neg30 = cpool.tile([P, 1], F32)
            nc.vector.memset(neg30[:], -30.0)

            CH = ((0, 64), (64, 128), (128, T))
            wkp_v = (
                wkp[:].rearrange("p (c two) -> p c two", c=C, two=2).unsqueeze(1)
            )

            def emit_load(i):
                """DMA keys chunks + ACT converts to interleaved bf16."""
                sl = slice(i * P, (i + 1) * P)
                kbf = kpool.tile([P, T * C], BF16, tag="kbf")
                kbf4 = kbf[:].rearrange(
                    "p (t2 c two) -> p t2 c two", t2=T2, c=C, two=2
                )
                for ci, (t0, t1) in enumerate(CH):
                    kfh = fpool.tile([P, 72 * C], F32, tag="kfh")
                    eng = nc.scalar if ci == 1 else nc.sync
                    eng.dma_start(
                        kfh[:, 0 : (t1 - t0) * C],
                        k_d[sl, t0:t1, :].rearrange("b t c -> b (t c)"),
                    )
                    src = kfh[:, 0 : (t1 - t0) * C].rearrange(
                        "p (t2 two c) -> p t2 c two", t2=(t1 - t0) // 2, two=2, c=C
                    )
                    nc.scalar.copy(kbf4[:, t0 // 2 : t1 // 2], src)
                q_t = spool.tile([P, C], F32, tag="q_t")
                nc.sync.dma_start(q_t[:], q_d[sl, 0, :])
                kl_t = spool.tile([P, 1], I32, tag="kl_t")
                nc.sync.dma_start(kl_t[:], kl_d[sl])
                return kbf, kbf4, q_t, kl_t

            def emit_smalls(ld):
                """DVE small ops: mask + query dot."""
                kbf, kbf4, q_t, kl_t = ld
                kl_f = spool.tile([P, 1], F32, tag="kl_f")
                nc.vector.tensor_copy(kl_f[:], kl_t[:])
                mask = spool.tile([P, T], F32, tag="mask")
                nc.vector.tensor_scalar(
                    mask[:], iota_f[:], kl_f[:], None, op0=mybir.AluOpType.is_lt
                )
                qprod = spool.tile([P, C], F32, tag="qprod")
                nc.vector.tensor_tensor(
                    qprod[:], q_t[:], wq_bc, op=mybir.AluOpType.mult
                )
                qdot = spool.tile([P, 1], F32, tag="qdot")
                nc.vector.reduce_sum(qdot[:], qprod[:], axis=mybir.AxisListType.X)
                qdotb = spool.tile([P, 1], F32, tag="qdotb")
                nc.vector.tensor_tensor(
                    qdotb[:], qdot[:], bias_bc, op=mybir.AluOpType.add
                )
                return mask, qdotb

            def emit_score(ld, sm_):
                """DVE prod1 + c-fold tree + kdot; ACT tanh."""
                kbf, kbf4, q_t, kl_t = ld
                mask, qdotb = sm_
                p1 = ppool.tile([P, T * C], BF16, tag="p1")
                p14 = p1[:].rearrange(
                    "p (t2 c two) -> p t2 c two", t2=T2, c=C, two=2
                )
                kdot = spool.tile([P, T], F32)
                for t0, t1 in CH:
                    a, b_ = t0 // 2, t1 // 2
                    n = b_ - a
                    nc.vector.tensor_tensor(
                        p14[:, a:b_],
                        kbf4[:, a:b_],
                        wkp_v.to_broadcast((P, n, C, 2)),
                        op=mybir.AluOpType.mult,
                    )
                    w_ = C // 2
                    while w_ >= 4:
                        nc.vector.tensor_tensor(
                            p14[:, a:b_, 0:w_],
                            p14[:, a:b_, 0:w_],
                            p14[:, a:b_, w_ : 2 * w_],
                            op=mybir.AluOpType.add,
                        )
                        w_ //= 2
                    red_in = p14[:, a:b_, 0:4].rearrange("p t2 c two -> p t2 two c")
                    nc.vector.reduce_sum(
                        kdot[:, 2 * a : 2 * b_], red_in, axis=mybir.AxisListType.X
                    )
                score = spool.tile([P, T], F32)
                nc.scalar.activation(
                    score[:],
                    kdot[:],
                    mybir.ActivationFunctionType.Tanh,
                    bias=qdotb[:],
                    scale=1.0,
                )
                return score

            def emit_output(i, ld, en):
                """DVE prod2 + t2-fold tree + out DMA."""
                sl = slice(i * P, (i + 1) * P)
                kbf, kbf4, q_t, kl_t = ld
                p2 = p2pool.tile([P, T * C], BF16, tag="p2")
                p24 = p2[:].rearrange(
                    "p (t2 c two) -> p t2 c two", t2=T2, c=C, two=2
                )
                en_v = (
                    en[:]
                    .rearrange("p (t2 two) -> p t2 two", t2=T2, two=2)
                    .unsqueeze(2)
                    .to_broadcast((P, T2, C, 2))
                )
                for a, b_ in ((0, 50), (50, T2)):
                    nc.vector.tensor_tensor(
                        p24[:, a:b_],
                        kbf4[:, a:b_],
                        en_v[:, a:b_],
                        op=mybir.AluOpType.mult,
                    )
                p2f = p2[:].rearrange("p (t2 cc) -> p t2 cc", t2=T2, cc=2 * C)
                for n, src in (
                    (50, 50),
                    (25, 25),
                    (12, 13),
                    (6, 7),
                    (3, 4),
                    (2, 2),
                    (1, 1),
                ):
                    nc.vector.tensor_tensor(
                        p2f[:, 0:n],
                        p2f[:, 0:n],
                        p2f[:, src : src + n],
                        op=mybir.AluOpType.add,
                    )
                out_t = spool.tile([P, C], F32)
                nc.vector.tensor_tensor(
                    out_t[:],
                    p24[:, 0, :, 0],
                    p24[:, 0, :, 1],
                    op=mybir.AluOpType.add,
                )
                nc.sync.dma_start(out_d[sl, 0, :], out_t[:])

            # ---- software-pipelined schedule ----
            # Converts run TWO tiles ahead (ACT block: exp_i, en_i,
            # conv_{i+2} x3, tanh_{i+1}) so DVE's score_{i+1} never waits
            # on ACT; DVE block: stt_i, smalls_{i+1}, recip_i, score_{i+1},
            # prod2_i, tfold_i.
            ld = {0: emit_load(0)}
            if N_TILES > 1:
                ld[1] = emit_load(1)
            sm_ = emit_smalls(ld[0])
            score = emit_score(ld[0], sm_)
            mask, _ = sm_
            for i in range(N_TILES):
                # stt_i: masked pre-exp logits
                sm_t = spool.tile([P, T], F32)
                nc.vector.scalar_tensor_tensor(
                    sm_t[:],
                    score[:],
                    30.0,
                    mask[:],
                    op0=mybir.AluOpType.add,
                    op1=mybir.AluOpType.mult,
                )
                e = spool.tile([P, T], F32)
                ssum = spool.tile([P, 1], F32)
                nc.scalar.activation(
                    e[:],
                    sm_t[:],
                    mybir.ActivationFunctionType.Exp,
                    bias=neg30[:],
                    scale=1.0,
                    accum_out=ssum[:],
                )
                if i + 1 < N_TILES:
                    sm_n = emit_smalls(ld[i + 1])  # DVE: fills ACT exp wait
                rs = spool.tile([P, 1], F32)
                nc.vector.reciprocal(rs[:], ssum[:])
                en = spool.tile([P, T], BF16)
                nc.scalar.activation(
                    en[:],
                    e[:],
                    mybir.ActivationFunctionType.Copy,
                    bias=0.0,
                    scale=rs[:],
                )
                if i + 2 < N_TILES:
                    ld[i + 2] = emit_load(i + 2)  # ACT convs after en_i
                if i + 1 < N_TILES:
                    # next tile's score on DVE fills the ACT en wait; its
                    # tanh on ACT lands after the i+2 converts
                    score = emit_score(ld[i + 1], sm_n)
                    mask, _ = sm_n
                emit_output(i, ld.pop(i), en)

    nc.compile()
    return nc


def get_kernel():
    if "nc" not in _NC_CACHE:
        _NC_CACHE["nc"] = build_kernel()
    return _NC_CACHE["nc"]


def kernel(queries, keys, keys_length, W, b, **run_kwargs):
    nc = get_kernel()
    in_maps = []
    for c in range(N_CORES):
        sl = slice(c * B, (c + 1) * B)
        in_maps.append(
            {
                "queries": np.ascontiguousarray(queries[sl], dtype=np.float32),
                "keys": np.ascontiguousarray(keys[sl], dtype=np.float32),
                "keys_length": np.ascontiguousarray(keys_length[sl], dtype=np.int32),
                "W": np.ascontiguousarray(W, dtype=np.float32),
                "b": np.ascontiguousarray(b, dtype=np.float32),
            }
        )
    res = run_bass_kernel_spmd(nc, in_maps, core_ids=list(range(N_CORES)), **run_kwargs)
    out = np.concatenate([res.results[c]["out"] for c in range(N_CORES)], axis=0)
    if run_kwargs:
        kernel.last_result = res
    return out
